# revision 41
# baseline (speedup 1.0000x reference)
"""Trainium2 Bass kernel for EvolveGCN-O forward (GCN message passing).

Math (reference):
    h   = x @ Wp + bp
    W   = LSTM-evolved weight from initial_weight (one step, h0=c0=IW)
    hw  = h @ W
    out = D^-1/2 (A+I) D^-1/2 hw + b_gcn

Factored for the kernel:
    out[d] = dinv[d] * (sum_src cnt[src,d] * dinv[src] * x[src]) @ (Wp @ W)
             + s2[d]*dinv[d]*(bp @ W) + b_gcn
with s2[d] = sum_{e in(d)} dinv[src_e] (self loops included as edges).

Strategy: nodes (dsts) block-sharded over 8 NeuronCores (1250 each, padded
to 1280). Instead of gathering per-edge source rows (SWDGE-descriptor
bound), the host builds a dense fp8 count matrix C [10240 src x 1280 dst]
per core; the aggregation is then a pure dense TensorEngine contraction
    xaggT[f, d] = sum_r Xs[r].T @ C[r, d-chunk]      (80 rank passes)
with Xs the dinv[src]-prescaled fp16 feature table (mixed fp16 x fp8
matmul). C streams from HBM fully overlapped with the PE. The tiny LSTM
weight evolution runs on-device in fp16, overlapped under the dense phase.
"""

import numpy as np
import ml_dtypes

N_NODES = 10000
N_EDGES = 320000
IN_DIM = 128
HID = 256
M = 8                    # NeuronCores
NP = 10240               # padded source count (mult of 128)
RANKS = NP // 128        # 80
NLOC = N_NODES // M      # 1250 dsts per core
NPC = 1280               # padded dsts per core
NGRP = NPC // 128        # 10 dst blocks of 128
CW = (512, 512, 256)     # dst chunk widths (PSUM bank = 512 fp32)
COFF = (0, RANKS * 512, RANKS * 1024)   # col offsets of chunks in cmat
CCOLS = RANKS * NPC      # 102400
RPG = 10                 # ranks per C dma group
NRG = RANKS // RPG       # 8 groups

_cache = {}


def _build_module():
    import concourse.bacc as bacc
    import concourse.mybir as mybir
    import concourse.tile as tile

    nc = bacc.Bacc("TRN2", target_bir_lowering=False, debug=False,
                   num_devices=M, num_swdge_queues=4)
    f32, f16, f8 = mybir.dt.float32, mybir.dt.float16, mybir.dt.float8e4

    # ---- DRAM inputs ----
    xs_in = nc.dram_tensor("xs", [128, NP], f16, kind="ExternalInput").ap()
    c_in = nc.dram_tensor("cmat", [128, CCOLS], f8, kind="ExternalInput").ap()
    wsum_in = nc.dram_tensor("wsumT", [256, 1024], f16, kind="ExternalInput").ap()
    bsum_in = nc.dram_tensor("bsum", [1, 1024], f16, kind="ExternalInput").ap()
    iw_in = nc.dram_tensor("IW", [256, 256], f32, kind="ExternalInput").ap()
    iwt_in = nc.dram_tensor("IWT", [256, 256], f16, kind="ExternalInput").ap()
    wpt_in = nc.dram_tensor("WpT", [256, 128], f16, kind="ExternalInput").ap()
    bp_in = nc.dram_tensor("bp_col", [256, 1], f16, kind="ExternalInput").ap()
    bgcn_in = nc.dram_tensor("b_gcn", [1, 256], f16, kind="ExternalInput").ap()
    ones_in = nc.dram_tensor("ones_row", [1, 128], f16, kind="ExternalInput").ap()
    sd_in = nc.dram_tensor("sd_rows", [2, NPC], f16, kind="ExternalInput").ap()
    dcol_in = nc.dram_tensor("dinv_col", [128, NGRP], f32, kind="ExternalInput").ap()

    out_t = nc.dram_tensor("out", [NPC, HID], f32, kind="ExternalOutput").ap()

    Sig = mybir.ActivationFunctionType.Sigmoid
    Tanh = mybir.ActivationFunctionType.Tanh
    Copy = mybir.ActivationFunctionType.Copy

    with tile.TileContext(nc) as tc:
        with (
            tc.tile_pool(name="pp", bufs=1) as pp,
            tc.tile_pool(name="cp", bufs=6) as cpool,
            tc.tile_pool(name="st", bufs=1) as stpool,
            tc.tile_pool(name="psacc", bufs=2, space="PSUM") as psacc,
            tc.tile_pool(name="psw", bufs=1, space="PSUM") as psw,
            tc.tile_pool(name="pst", bufs=3, space="PSUM") as pst,
        ):
            # ---------- streaming loads (sync queue): xs interleaved with C --
            # 10-rank granules; tile free width equals the chunk width so the
            # SBUF write stays contiguous (a 256-wide slice of a 512-wide
            # tile fragments the DMA into per-256B descriptors)
            ct = {}
            xs_sb = pp.tile([128, NP], f16)

            def emit_c_load(c, r0, nr):
                w = CW[c]
                t = cpool.tile([128, nr, w], f8, tag=f"ct{w}_{nr}")
                nc.sync.dma_start(
                    out=t[:],
                    in_=c_in[:, COFF[c] + r0 * w:COFF[c] + (r0 + nr) * w]
                        .rearrange("p (r j) -> p r j", j=w))
                for i in range(nr):
                    ct[(c, r0 + i)] = (t, i)

            # small granules through rank 30 (the choppy early DMA-delivery
            # window where issue-serialized 10-rank granules trail the PE by
            # ~0.2us each); 10-rank granules once the pipeline is ahead
            C0_GRAN = ([(5 * k, 5) for k in range(6)]
                       + [(30 + 10 * k, 10) for k in range(5)])
            for r0, nr in C0_GRAN:
                nc.sync.dma_start(
                    out=xs_sb[:, r0 * 128:(r0 + nr) * 128],
                    in_=xs_in[:, r0 * 128:(r0 + nr) * 128])
                emit_c_load(0, r0, nr)
            for rg in range(NRG):
                emit_c_load(1, rg * RPG, RPG)
            for rg in range(4):
                emit_c_load(2, rg * 20, 20)

            # ---------- small tensors: scalar queue ----------
            wsum = pp.tile([128, 2, 1024], f16)
            iwt = pp.tile([128, 2, 256], f16)
            iw = pp.tile([128, 2, 256], f32)
            wpt = pp.tile([128, 2, 128], f16)
            bp_c = pp.tile([128, 2, 1], f16)
            bsum = pp.tile([1, 1024], f16)
            ones = pp.tile([1, 128], f16)
            bb = pp.tile([2, 256], f16)      # rows: [bpw (computed), b_gcn]
            sd = pp.tile([2, NPC], f16)      # rows: [s2, 1/dinv]
            dcol = pp.tile([128, NGRP], f32)
            for t_, s_ in ((wsum, wsum_in), (iwt, iwt_in), (iw, iw_in),
                           (wpt, wpt_in), (bp_c, bp_in)):
                nc.scalar.dma_start(out=t_[:], in_=s_.rearrange("(k p) c -> p k c", p=128))
            for t_, s_ in ((bsum, bsum_in), (ones, ones_in), (sd, sd_in),
                           (dcol, dcol_in)):
                nc.scalar.dma_start(out=t_[:], in_=s_[:])
            nc.scalar.dma_start(out=bb[1:2, :], in_=bgcn_in[:])

            # ---------- PE warm-up (p-state ramp while DMAs land) ----------
            warm = pp.tile([128, 384], f16)
            nc.vector.memset(warm[:], 0.0)
            wps = pst.tile([128, HID], f32, space="PSUM", tag="ops")
            for i in range(32):
                nc.tensor.matmul(out=wps[:], lhsT=warm[:, :128], rhs=warm[:, 128:384],
                                 start=(i == 0), stop=(i == 31),
                                 skip_group_check=True)

            xaggT = pp.tile([128, NPC], f16)
            accs = {}

            def dense_part(c, r_lo, r_hi, hooks=None):
                w = CW[c]
                if c not in accs:
                    acc = psacc.tile([128, 512], f32, space="PSUM", tag="acc")
                    accs[c] = acc
                acc = accs[c]
                for gr in range(r_lo, r_hi):
                    t, i = ct[(c, gr)]
                    nc.tensor.matmul(
                        out=acc[:, :w],
                        lhsT=xs_sb[:, gr * 128:(gr + 1) * 128],
                        rhs=t[:, i, :w],
                        start=(gr == 0), stop=(gr == RANKS - 1),
                        skip_group_check=True)
                    if hooks and gr in hooks:
                        hooks[gr]()

            def chunk_copy(c):
                w = CW[c]
                nc.scalar.activation(out=xaggT[:, c * 512:c * 512 + w],
                                     in_=accs[c][:, :w], func=Copy)

            w_ev = pp.tile([128, 2, 256], f16)

            def lstm_gates(ic):
                g = psw.tile([128, 1024], f32, space="PSUM", tag="gates")
                for h in range(2):
                    gs = slice(512 * h, 512 * (h + 1))
                    nc.tensor.matmul(out=g[:, gs], lhsT=ones[:, :],
                                     rhs=bsum[:, gs], start=True, stop=False,
                                     skip_group_check=True)
                    nc.tensor.matmul(out=g[:, gs],
                                     lhsT=iwt[:, 0, 128 * ic:128 * (ic + 1)],
                                     rhs=wsum[:, 0, gs], start=False, stop=False,
                                     skip_group_check=True)
                    nc.tensor.matmul(out=g[:, gs],
                                     lhsT=iwt[:, 1, 128 * ic:128 * (ic + 1)],
                                     rhs=wsum[:, 1, gs], start=False, stop=True,
                                     skip_group_check=True)
                si = stpool.tile([128, 256], f32, tag=f"si{ic}")
                sf = stpool.tile([128, 256], f32, tag=f"sf{ic}")
                tg = stpool.tile([128, 256], f32, tag=f"tg{ic}")
                so = stpool.tile([128, 256], f32, tag=f"so{ic}")
                # si/tg first: the vector chain's critical path starts at c2
                nc.scalar.activation(out=si[:], in_=g[:, 0:256], func=Sig)
                nc.scalar.activation(out=tg[:], in_=g[:, 512:768], func=Tanh)
                nc.scalar.activation(out=sf[:], in_=g[:, 256:512], func=Sig)
                nc.scalar.activation(out=so[:], in_=g[:, 768:1024], func=Sig)
                c2 = stpool.tile([128, 256], f32, tag=f"c2{ic}")
                nc.vector.tensor_tensor(out=c2[:], in0=si[:], in1=tg[:],
                                        op=mybir.AluOpType.mult)
                c1 = stpool.tile([128, 256], f32, tag=f"c1{ic}")
                nc.vector.tensor_tensor(out=c1[:], in0=sf[:], in1=iw[:, ic, :],
                                        op=mybir.AluOpType.mult)
                cc = stpool.tile([128, 256], f32, tag=f"cc{ic}")
                nc.vector.tensor_tensor(out=cc[:], in0=c1[:], in1=c2[:],
                                        op=mybir.AluOpType.add)
                tcc = stpool.tile([128, 256], f32, tag=f"tcc{ic}")
                nc.scalar.activation(out=tcc[:], in_=cc[:], func=Tanh)
                nc.vector.tensor_tensor(out=w_ev[:, ic, :], in0=so[:],
                                        in1=tcc[:], op=mybir.AluOpType.mult)

            wpw = pp.tile([128, 256], f16)

            def lstm_proj():
                wb = psw.tile([128, 512], f32, space="PSUM", tag="wb")
                nc.tensor.matmul(out=wb[:, :256], lhsT=wpt[:, 0, :], rhs=w_ev[:, 0, :],
                                 start=True, stop=False, skip_group_check=True)
                nc.tensor.matmul(out=wb[:, :256], lhsT=wpt[:, 1, :], rhs=w_ev[:, 1, :],
                                 start=False, stop=True, skip_group_check=True)
                nc.scalar.activation(out=wpw[:], in_=wb[:, :256], func=Copy)
                nc.tensor.matmul(out=wb[:1, 256:512], lhsT=bp_c[:, 0, :],
                                 rhs=w_ev[:, 0, :], start=True, stop=False,
                                 skip_group_check=True)
                nc.tensor.matmul(out=wb[:1, 256:512], lhsT=bp_c[:, 1, :],
                                 rhs=w_ev[:, 1, :], start=False, stop=True,
                                 skip_group_check=True)
                nc.scalar.activation(out=bb[0:1, :], in_=wb[:1, 256:512], func=Copy)

            def tail(g):
                ops = pst.tile([128, HID], f32, space="PSUM", tag="ops")
                ds = slice(128 * g, 128 * (g + 1))
                nc.tensor.matmul(out=ops[:], lhsT=sd[:, ds], rhs=bb[:],
                                 start=True, stop=False, skip_group_check=True)
                nc.tensor.matmul(out=ops[:], lhsT=xaggT[:, ds], rhs=wpw[:],
                                 start=False, stop=True, skip_group_check=True)
                orow = stpool.tile([128, HID], f32, tag=f"orow{g % 3}")
                if g % 2 == 0:
                    nc.scalar.activation(out=orow[:], in_=ops[:], func=Copy,
                                         scale=dcol[:, g:g + 1])
                else:
                    # odd blocks scale on the vector engine so the 10 output
                    # copies don't serialize on the scalar queue
                    nc.vector.tensor_tensor(
                        out=orow[:], in0=ops[:],
                        in1=dcol[:, g:g + 1].to_broadcast([128, HID]),
                        op=mybir.AluOpType.mult)
                nc.gpsimd.dma_start(
                    out=out_t.rearrange("(g p) h -> g p h", p=128)[g],
                    in_=orow[:])

            # ---------- PE program ----------
            # proj hooks sit deep enough in chunk 2 that the scalar/vector
            # activation chain producing w_ev (≈3.5us after gates1) is done
            dense_part(0, 0, 80)
            chunk_copy(0)
            lstm_gates(0)
            dense_part(1, 0, 80)
            chunk_copy(1)
            lstm_gates(1)
            hooks = {
                45: lstm_proj,
                52: lambda: (tail(0), tail(1)),
                58: lambda: (tail(2), tail(3)),
                64: lambda: (tail(4), tail(5)),
                70: lambda: (tail(6), tail(7)),
            }
            dense_part(2, 0, 80, hooks=hooks)
            chunk_copy(2)
            tail(8)
            tail(9)

    nc.compile()
    return nc


def _preprocess(x, edge_index):
    """Host-side structure preprocessing: count matrix + degree vectors."""
    src = np.asarray(edge_index[0], dtype=np.int64)
    dst = np.asarray(edge_index[1], dtype=np.int64)
    loops = np.arange(N_NODES, dtype=np.int64)
    src_all = np.concatenate([src, loops])
    dst_all = np.concatenate([dst, loops])
    deg = np.bincount(dst_all, minlength=N_NODES).astype(np.float64)
    dinv = (1.0 / np.sqrt(deg)).astype(np.float32)

    # dense per-core count matrices, chunk-major fp8 layout
    core = dst_all // NLOC
    dl = dst_all - core * NLOC
    ch = dl >> 9                      # dl // 512 (0,1,2)
    j = dl - (ch << 9)
    r = src_all >> 7
    p = src_all & 127
    cwv = np.array(CW, np.int64)
    offv = np.array(COFF, np.int64)
    col = offv[ch] + r * cwv[ch] + j
    cnt = np.zeros((M, 128, CCOLS), np.uint8)
    np.add.at(cnt, (core, p, col), 1)
    lut = np.arange(256, dtype=np.float32).astype(ml_dtypes.float8_e4m3)
    cmat = lut[cnt]

    # s2[d] = sum over in-edges of dinv[src] (self loop included)
    s2 = np.bincount(dst_all, weights=dinv[src_all].astype(np.float64),
                     minlength=N_NODES).astype(np.float32)

    # dinv[src]-prescaled fp16 feature table, token layout [p=n%128, n//128*128+f]
    xp = np.zeros((NP, IN_DIM), np.float32)
    xp[:N_NODES] = x * dinv[:, None]
    xs_t = np.ascontiguousarray(
        xp.reshape(RANKS, 128, IN_DIM).transpose(1, 0, 2).reshape(128, NP)
    ).astype(np.float16)

    return dict(dinv=dinv, cmat=cmat, s2=s2, xs_t=xs_t)


LAST_RESULT = None


def kernel(x, edge_index, Wp, bp, W_ih, W_hh, b_ih, b_hh, initial_weight, b_gcn):
    global LAST_RESULT
    from concourse.bass_utils import run_bass_kernel_spmd

    x = np.asarray(x, np.float32)
    Wp = np.asarray(Wp, np.float32)
    bp = np.asarray(bp, np.float32)
    W_ih = np.asarray(W_ih, np.float32)
    W_hh = np.asarray(W_hh, np.float32)
    b_ih = np.asarray(b_ih, np.float32)
    b_hh = np.asarray(b_hh, np.float32)
    initial_weight = np.asarray(initial_weight, np.float32)
    b_gcn = np.asarray(b_gcn, np.float32)
    assert x.shape == (N_NODES, IN_DIM)

    pre = _preprocess(x, edge_index)
    dinv, s2 = pre["dinv"], pre["s2"]

    if "m" not in _cache:
        _cache["m"] = _build_module()
    nc = _cache["m"]

    shared = {
        "xs": pre["xs_t"],
        "wsumT": np.ascontiguousarray((W_ih + W_hh).T).astype(np.float16),
        "bsum": (b_ih + b_hh).reshape(1, -1).astype(np.float16),
        "IW": initial_weight,
        "IWT": np.ascontiguousarray(initial_weight.T).astype(np.float16),
        "WpT": np.ascontiguousarray(Wp.T).astype(np.float16),
        "bp_col": np.ascontiguousarray(bp.reshape(-1, 1)).astype(np.float16),
        "b_gcn": b_gcn.reshape(1, -1).astype(np.float16),
        "ones_row": np.ones((1, 128), np.float16),
    }
    in_maps = []
    for c in range(M):
        lo, hi = c * NLOC, (c + 1) * NLOC
        sdp = np.zeros((2, NPC), np.float16)
        sdp[0, :NLOC] = s2[lo:hi]
        sdp[1, :NLOC] = 1.0 / dinv[lo:hi]
        dlocp = np.zeros(NPC, np.float32)
        dlocp[:NLOC] = dinv[lo:hi]
        in_maps.append({
            **shared,
            "cmat": np.ascontiguousarray(pre["cmat"][c]),
            "sd_rows": sdp,
            "dinv_col": np.ascontiguousarray(dlocp.reshape(NGRP, 128).T),
        })

    res = run_bass_kernel_spmd(nc, in_maps, list(range(M)))
    LAST_RESULT = res

    out = np.empty((N_NODES, HID), np.float32)
    for c in range(M):
        out[c * NLOC:(c + 1) * NLOC] = res.results[c]["out"][:NLOC]
    return out
